# revision 5
# baseline (speedup 1.0000x reference)
"""GATv2 GNN classifier (nn_AttGNNClassifier) as an 8-core Trainium2 Bass kernel.

Strategy (graph-parallel, v2):
  - Nodes are partitioned contiguously across 8 cores; within a core they are
    degree-balance packed into NT=49 tiles of 128 (snake packing), so per-tile
    edge counts are near-uniform and padding is minimal.
  - The fs table rows are laid out AG-chunk-major so the per-layer AllGather
    can be issued in 4 chunks that pipeline under projection compute.
  - Edges are assigned to the (core, tile) owning their dst node, split into
    two sections by table row (< / >= 32768 so gather indices fit int16), and
    sorted by src row inside each section for HBM locality. Per-tile section
    sizes are ragged (padded only to 128).
  - All one-hot matrices (dst scatter in both major orders, graph selector)
    are precomputed on the host as fp8 and streamed from DRAM via HWDGE,
    eliminating the on-device DVE is_equal builds and gpsimd broadcasts.
  - Per layer: project fs/fd per tile, all-gather fs chunks, then per tile:
    dma_gather fs[src] rows, z = fd[dst]+fs via one-hot+identity matmuls in
    PSUM, leaky_relu, logits (DVE mult+reduce), exp, and a one-hot matmul
    computing weighted sums and softmax denominators in one PSUM pass.
  - h is written to DRAM and re-loaded transposed via HWDGE transpose-DMA for
    the next layer's projection; projection+AG for layer l+1 are interleaved
    into layer l's edge loop per AG chunk (software pipeline).
  - Graph mean-pool via fp8 one-hot matmul accumulated across tiles, an
    all-reduce of [G, 65] partials, then the tiny pattern/classifier MLP on
    every core; core 0's output is returned.
"""

import math

import ml_dtypes
import numpy as np

import concourse.bass as bass
import concourse.bacc as bacc
import concourse.mybir as mybir
import concourse.tile as tile
from concourse import library_config
from concourse.bass_utils import run_bass_kernel_spmd

F16 = mybir.dt.float16
F32 = mybir.dt.float32
F8 = mybir.dt.float8e4
I16 = mybir.dt.int16

NEG_GAT = 0.2
NEG = 0.01


def _default_cfg():
    return dict(
        NC=8, N=50000, E=400000, F_IN=128, H=3, D=64, G=64, P=64, SPLIT=32768,
    )


def _derive(cfg):
    c = dict(cfg)
    c["HD"] = c["H"] * c["D"]
    c["TE"] = 256                      # table row elems (512B rows, fp16)
    c["NPC"] = 49 * 128                # nodes per core, padded
    c["NPAD"] = c["NC"] * c["NPC"]
    c["NT"] = 49
    c["NPC_REAL"] = c["N"] // c["NC"]  # 6250
    c["CHUNKS"] = [(0, 13), (13, 25), (25, 37), (37, 49)]
    c["HPAD"] = 256                    # h DRAM row elems (for transpose-DMA)
    assert c["F_IN"] <= 128
    return c


# ---------------------------------------------------------------- host prep

def _wrap16(vals, F):
    """int16 values -> [128, F] wrapped (k -> [k%16, k//16]) x8 replicated."""
    out = np.zeros((128, F), np.int16)
    k = np.arange(len(vals))
    out[k % 16, k // 16] = vals
    for g in range(1, 8):
        out[16 * g : 16 * g + 16] = out[:16]
    return out


def _pack_nodes(deg, NT):
    """Snake-pack node indices (local) into NT bins of <=128 by degree desc.

    Returns (bin_of_node, slot_of_node)."""
    n = len(deg)
    order = np.argsort(-deg, kind="stable")
    bin_of = np.empty(n, np.int64)
    rounds = (n + NT - 1) // NT
    pos = 0
    for r in range(rounds):
        take = min(NT, n - pos)
        idx = order[pos : pos + take]
        if r % 2 == 0:
            bins = np.arange(take)
        else:
            bins = NT - 1 - np.arange(take)
        bin_of[idx] = bins
        pos += take
    # slots: order of assignment within bin (stable by round)
    slot_of = np.empty(n, np.int64)
    counts = np.zeros(NT, np.int64)
    pos = 0
    for r in range(rounds):
        take = min(NT, n - pos)
        idx = order[pos : pos + take]
        b = bin_of[idx]
        slot_of[idx] = counts[b]
        counts[b] += 1
        pos += take
    assert counts.max() <= 128
    return bin_of, slot_of


def prep_host(inputs, cfg):
    c = cfg
    NC, N, NPC, NT, G = c["NC"], c["N"], c["NPC"], c["NT"], c["G"]
    H, D, HD, F_IN, TE, SPLIT = c["H"], c["D"], c["HD"], c["F_IN"], c["TE"], c["SPLIT"]
    NPR = c["NPC_REAL"]
    CHUNKS = c["CHUNKS"]

    src = np.asarray(inputs["src"]).astype(np.int64)
    dst = np.asarray(inputs["dst"]).astype(np.int64)
    graph_ids = np.asarray(inputs["graph_ids"]).astype(np.int64)
    x = np.asarray(inputs["inputs"]).astype(np.float32)

    # --- node -> (core, tile, slot) assignment (degree-balanced) ---
    core_of = np.minimum(np.arange(N) // NPR, NC - 1)
    deg = np.bincount(dst, minlength=N)
    tile_of = np.empty(N, np.int64)
    slot_of = np.empty(N, np.int64)
    for co in range(NC):
        lo, hi = co * NPR, (co + 1) * NPR
        b, s = _pack_nodes(deg[lo:hi], NT)
        tile_of[lo:hi] = b
        slot_of[lo:hi] = s

    # --- table row mapping (AG chunk-major) ---
    # chunk c covers tiles [t0,t1); rows_c = (t1-t0)*128
    # row(core, tile, slot) = base_c + core*rows_c + (tile-t0)*128 + slot
    chunk_of_tile = np.empty(NT, np.int64)
    tile_base = np.empty(NT, np.int64)  # row offset of (tile,0) within a core's chunk stripe
    chunk_base = []
    base = 0
    for ci, (t0, t1) in enumerate(CHUNKS):
        rows_c = (t1 - t0) * 128
        chunk_base.append(base)
        for t in range(t0, t1):
            chunk_of_tile[t] = ci
            tile_base[t] = (t - t0) * 128
        base += NC * rows_c
    chunk_base = np.asarray(chunk_base)
    chunk_rows = np.asarray([(t1 - t0) * 128 for (t0, t1) in CHUNKS])

    def rows_of(core, tiles, slots):
        ci = chunk_of_tile[tiles]
        return chunk_base[ci] + core * chunk_rows[ci] + tile_base[tiles] + slots

    row_of_node = rows_of(core_of, tile_of[np.arange(N)], slot_of[np.arange(N)])

    # --- edge partitioning ---
    e_core = core_of[dst]
    e_tile = tile_of[dst]
    e_slot = slot_of[dst]          # dst slot within tile
    e_srcrow = row_of_node[src]
    e_sect = (e_srcrow >= SPLIT).astype(np.int64)

    key = ((e_core * NT + e_tile) * 2 + e_sect)
    order = np.lexsort((e_srcrow, key))  # sorted by key, then srcrow
    cnt = np.bincount(key, minlength=NC * NT * 2).reshape(NC, NT, 2)
    eca = np.maximum(1, np.ceil(cnt[:, :, 0].max(axis=0) / 128).astype(int))  # [NT]
    ecb = np.ceil(cnt[:, :, 1].max(axis=0) / 128).astype(int)                 # [NT]
    KA = eca * 128
    KB = ecb * 128
    ET = KA + KB
    EC = ET // 128
    ECMAX = int(EC.max())

    # per-tile free-dim offsets in the concatenated DRAM tensors
    offA = np.concatenate([[0], np.cumsum(KA // 16)])
    offB = np.concatenate([[0], np.cumsum(KB // 16)])
    offO = np.concatenate([[0], np.cumsum(ET)])
    FA_TOT, FB_TOT, O_TOT = int(offA[-1]), int(offB[-1]), int(offO[-1])

    idxA = np.zeros((NC, 128, FA_TOT), np.int16)
    idxB = np.zeros((NC, 128, max(FB_TOT, 1)), np.int16)
    snm = np.zeros((NC, 128, O_TOT), np.float32)
    stt = np.zeros((NC, 128, O_TOT), np.float32)

    starts = np.concatenate([[0], np.cumsum(cnt.reshape(-1))]).astype(np.int64)
    for co in range(NC):
        for t in range(NT):
            for s in range(2):
                k = (co * NT + t) * 2 + s
                lo, hi = starts[k], starts[k + 1]
                e = order[lo:hi]
                n = hi - lo
                kpad = KA[t] if s == 0 else KB[t]
                assert n <= kpad, (co, t, s, n, kpad)
                base_s = 0 if s == 0 else KA[t]
                if s == 0:
                    v = np.zeros(kpad, np.int64)
                    v[:n] = e_srcrow[e]
                    idxA[co, :, offA[t] : offA[t + 1]] = _wrap16(v, kpad // 16)
                else:
                    if kpad:
                        v = np.zeros(kpad, np.int64)
                        v[:n] = e_srcrow[e] - SPLIT
                        idxB[co, :, offB[t] : offB[t + 1]] = _wrap16(v, kpad // 16)
                if n:
                    sl = base_s + np.arange(n)
                    p, j = sl % 128, sl // 128
                    snm[co, e_slot[e], offO[t] + sl] = 1.0
                    stt[co, p, offO[t] + j * 128 + e_slot[e]] = 1.0

    # graph selector one-hots [128, NT*G]
    gsel = np.zeros((NC, 128, NT * G), np.float32)
    nid = np.arange(N)
    gsel[core_of, slot_of, tile_of * G + graph_ids[nid]] = 1.0

    # x feature-major per core, packed node order
    x_fm = np.zeros((NC, F_IN, NPC), np.float16)
    colv = tile_of * 128 + slot_of
    for co in range(NC):
        m = core_of == co
        x_fm[co][:, colv[m]] = x[m].T.astype(np.float16)

    rep = lambda v, p=128: np.broadcast_to(
        np.asarray(v, np.float16)[None, :], (p, len(v))
    ).copy()

    def w16(k):
        return np.asarray(inputs[k]).astype(np.float16)

    def ws_pad(k):  # pad output cols HD -> TE
        w = np.asarray(inputs[k]).astype(np.float16)
        out = np.zeros((w.shape[0], TE), np.float16)
        out[:, :HD] = w
        return out

    a_flat = [np.asarray(inputs[f"a{l}"]).astype(np.float32).reshape(-1) for l in (1, 2, 3)]
    b_flat = [np.asarray(inputs[f"b{l}"]).astype(np.float32) for l in (1, 2, 3)]
    b3m = b_flat[2].reshape(H, D).mean(0)

    FP8 = ml_dtypes.float8_e4m3
    ident8 = np.eye(128, dtype=np.float32).astype(FP8)
    ident = np.eye(128, dtype=np.float16)

    bex = np.asarray(inputs["bex"]).astype(np.float32)
    bex96 = np.concatenate([bex, bex, bex])

    common = dict(
        W1s=ws_pad("W1s"), W1d=w16("W1d"),
        W2s=ws_pad("W2s"), W2d=w16("W2d"),
        W3s=ws_pad("W3s"), W3d=w16("W3d"),
        a1_rep=rep(a_flat[0]), a2_rep=rep(a_flat[1]), a3_rep=rep(a_flat[2]),
        b1_rep=rep(b_flat[0]), b2_rep=rep(b_flat[1]),
        b3m_rep=rep(b3m),
        ident8=ident8, ident=ident,
        p1T=w16("p1").T.copy(), p2T=w16("p2").T.copy(), p3T=w16("p3").T.copy(),
        Wex=w16("Wex"), bex96_rep=rep(bex96, c["G"]),
        Wpat=w16("Wpat"), bpat_rep=rep(np.asarray(inputs["bpat"], np.float32), G),
        Wc1=w16("Wc1"), bc1_rep=rep(np.asarray(inputs["bc1"], np.float32), G),
        Wc2=w16("Wc2"), bc2_rep=rep(np.asarray(inputs["bc2"], np.float32), G),
        Wc3=w16("Wc3"), bc3_rep=rep(np.asarray(inputs["bc3"], np.float32), G),
    )

    in_maps = []
    for co in range(NC):
        m = dict(common)
        m["x_fm"] = x_fm[co]
        m["idxA"] = idxA[co]
        m["idxB"] = idxB[co]
        m["snm_all"] = snm[co].astype(FP8)
        m["st_all"] = stt[co].astype(FP8)
        m["gsel_all"] = gsel[co].astype(FP8)
        in_maps.append(m)

    meta = dict(
        eca=eca.tolist(), ecb=ecb.tolist(), EC=EC.tolist(),
        offA=offA.tolist(), offB=offB.tolist(), offO=offO.tolist(),
        FA_TOT=FA_TOT, FB_TOT=max(FB_TOT, 1), O_TOT=O_TOT, ECMAX=ECMAX,
        chunk_base=chunk_base.tolist(), chunk_rows=chunk_rows.tolist(),
    )
    return in_maps, meta


# ---------------------------------------------------------------- device build

def build_gat(cfg, meta):
    c = cfg
    NC, NPC, NPAD, NT, G = c["NC"], c["NPC"], c["NPAD"], c["NT"], c["G"]
    H, D, HD, F_IN, TE, SPLIT = c["H"], c["D"], c["HD"], c["F_IN"], c["TE"], c["SPLIT"]
    HPAD = c["HPAD"]
    CHUNKS = c["CHUNKS"]
    eca, ecb, ECl = meta["eca"], meta["ecb"], meta["EC"]
    offA, offB, offO = meta["offA"], meta["offB"], meta["offO"]
    ECMAX = meta["ECMAX"]
    chunk_base, chunk_rows = meta["chunk_base"], meta["chunk_rows"]

    nc = bacc.Bacc("TRN2", target_bir_lowering=False, debug=False, num_devices=NC,
                   num_swdge_queues=4)

    def din(name, shape, dt=F16):
        return nc.dram_tensor(name, shape, dt, kind="ExternalInput")

    x_fm = din("x_fm", [F_IN, NPC])
    idxA = din("idxA", [128, meta["FA_TOT"]], I16)
    idxB = din("idxB", [128, meta["FB_TOT"]], I16)
    snm_all = din("snm_all", [128, meta["O_TOT"]], F8)
    st_all = din("st_all", [128, meta["O_TOT"]], F8)
    gsel_all = din("gsel_all", [128, NT * G], F8)

    Wmat = {
        1: (din("W1s", [F_IN, TE]), din("W1d", [F_IN, HD])),
        2: (din("W2s", [HD, TE]), din("W2d", [HD, HD])),
        3: (din("W3s", [HD, TE]), din("W3d", [HD, HD])),
    }
    a_rep = {l: din(f"a{l}_rep", [128, HD]) for l in (1, 2, 3)}
    b_rep = {1: din("b1_rep", [128, HD]), 2: din("b2_rep", [128, HD])}
    b3m_rep = din("b3m_rep", [128, D])
    ident8_d = din("ident8", [128, 128], F8)
    ident_d = din("ident", [128, 128])
    p123T = [din("p1T", [64, G]), din("p2T", [64, G]), din("p3T", [64, G])]
    Wex = din("Wex", [64, 32])
    bex96_rep = din("bex96_rep", [G, 96])
    Wpat = din("Wpat", [96, 64])
    bpat_rep = din("bpat_rep", [G, 64])
    Wc1 = din("Wc1", [128, 64])
    bc1_rep = din("bc1_rep", [G, 64])
    Wc2 = din("Wc2", [64, 32])
    bc2_rep = din("bc2_rep", [G, 32])
    Wc3 = din("Wc3", [32, 2])
    bc3_rep = din("bc3_rep", [G, 2])

    out = nc.dram_tensor("out", [G, 2], F32, kind="ExternalOutput")

    # per (layer, chunk) internal DRAM
    fs_own = {
        (l, ci): nc.dram_tensor(f"fs_own{l}_{ci}", [NC_rows, TE], F16)
        for l in (1, 2, 3)
        for ci, NC_rows in enumerate(chunk_rows)
    }
    h_dram = {
        (l, ci): nc.dram_tensor(f"h{l}_{ci}", [chunk_rows[ci], HPAD], F16)
        for l in (1, 2)
        for ci in range(len(CHUNKS))
    }
    fs_full = {
        l: nc.dram_tensor(f"fs_full{l}", [NPAD, TE], F16, addr_space="Shared")
        for l in (1, 2, 3)
    }
    partials = nc.dram_tensor("partials", [G, 65], F32)
    partials_red = nc.dram_tensor("partials_red", [G, 65], F32, addr_space="Shared")

    groups = [list(range(NC))]
    FCH = [(0, 128), (128, 64)]  # lhs chunks for HD=192 contraction

    with tile.TileContext(nc) as tc:
        with (
            tc.tile_pool(name="const", bufs=1) as cpool,
            tc.tile_pool(name="wpool", bufs=1) as wpool,
            tc.tile_pool(name="hT", bufs=2) as hTpool,
            tc.tile_pool(name="proj", bufs=3) as ppool,
            tc.tile_pool(name="edge", bufs=2) as epool,
            tc.tile_pool(name="gath", bufs=3) as gpool,
            tc.tile_pool(name="oneh", bufs=3) as opool,
            tc.tile_pool(name="small", bufs=2) as spool,
            tc.tile_pool(name="psA", bufs=2, space="PSUM") as psA,
            tc.tile_pool(name="psZ", bufs=2, space="PSUM") as psZ,
            tc.tile_pool(name="psB", bufs=2, space="PSUM") as psB,
            tc.tile_pool(name="psT", bufs=1, space="PSUM") as psT,
            tc.tile_pool(name="psG", bufs=1, space="PSUM") as psG,
        ):
            nc.gpsimd.load_library(library_config.mlp)

            # ---------- resident constants
            ident8_t = cpool.tile([128, 128], F8)
            nc.sync.dma_start(ident8_t[:], ident8_d[:])
            ident_t = cpool.tile([128, 128], F16)
            nc.sync.dma_start(ident_t[:], ident_d[:])
            a_t = {l: cpool.tile([128, HD], F16, tag=f"a{l}", name=f"a{l}_t") for l in (1, 2, 3)}
            for l in (1, 2, 3):
                nc.sync.dma_start(a_t[l][:], a_rep[l][:])
            b_t = {l: cpool.tile([128, HD], F16, tag=f"b{l}", name=f"b{l}_t") for l in (1, 2)}
            for l in (1, 2):
                nc.sync.dma_start(b_t[l][:], b_rep[l][:])
            b3m_t = cpool.tile([128, D], F16)
            nc.sync.dma_start(b3m_t[:], b3m_rep[:])
            x_fm_t = cpool.tile([F_IN, NPC], F16)
            nc.sync.dma_start(x_fm_t[:], x_fm[:])
            idxA_t = cpool.tile([128, meta["FA_TOT"]], I16)
            nc.sync.dma_start(idxA_t[:], idxA[:])
            idxB_t = cpool.tile([128, meta["FB_TOT"]], I16)
            nc.sync.dma_start(idxB_t[:], idxB[:])
            gsel_t = cpool.tile([128, NT * G], F8)
            nc.sync.dma_start(gsel_t[:], gsel_all[:])

            Wt = {}
            for l in (1, 2, 3):
                kdim = F_IN if l == 1 else HD
                chs = [(0, kdim)] if kdim <= 128 else FCH
                Wt[l] = []
                for ci, (off, sz) in enumerate(chs):
                    ws = wpool.tile([sz, TE], F16, tag=f"W{l}s{ci}", name=f"W{l}s{ci}_t")
                    wd = wpool.tile([sz, HD], F16, tag=f"W{l}d{ci}", name=f"W{l}d{ci}_t")
                    nc.sync.dma_start(ws[:], Wmat[l][0][off : off + sz, :])
                    nc.sync.dma_start(wd[:], Wmat[l][1][off : off + sz, :])
                    Wt[l].append((ws, wd))

            fd_res = [
                cpool.tile([128, NT, HD], F16, tag=f"fd{i}", name=f"fd_res{i}")
                for i in (0, 1)
            ]
            fd_of = {1: fd_res[0], 2: fd_res[1], 3: fd_res[0]}

            gp_ps = psG.tile([G, 65], F32, space="PSUM")

            def proj_tile(l, lhs_chunks, tloc, ci):
                """Project one 128-node column group for layer l; writes
                fs_own[(l, ci)] rows and fd_of[l][:, t_global, :]."""
                t0 = CHUNKS[ci][0]
                tg = t0 + tloc
                ps_fs = psA.tile([128, TE], F32, space="PSUM", tag="psP", name="ps_fs")
                ps_fd = psA.tile([128, HD], F32, space="PSUM", tag="psP", name="ps_fd")
                for k, lt in enumerate(lhs_chunks):
                    nc.tensor.matmul(
                        ps_fs[:], lhsT=lt, rhs=Wt[l][k][0][:],
                        start=(k == 0), stop=(k == len(lhs_chunks) - 1),
                    )
                for k, lt in enumerate(lhs_chunks):
                    nc.tensor.matmul(
                        ps_fd[:], lhsT=lt, rhs=Wt[l][k][1][:],
                        start=(k == 0), stop=(k == len(lhs_chunks) - 1),
                    )
                fs_sb = ppool.tile([128, TE], F16, tag="fs_sb")
                nc.scalar.copy(fs_sb[:], ps_fs[:])
                nc.scalar.copy(fd_of[l][:, tg, :], ps_fd[:])
                nc.sync.dma_start(
                    fs_own[(l, ci)][bass.ts(tloc, 128), :], fs_sb[:]
                )

            def ag_chunk(l, ci):
                rows = chunk_rows[ci]
                nc.gpsimd.collective_compute(
                    "AllGather",
                    mybir.AluOpType.bypass,
                    replica_groups=groups,
                    ins=[fs_own[(l, ci)][:].rearrange("a b -> (a b)")],
                    outs=[
                        fs_full[l][
                            chunk_base[ci] : chunk_base[ci] + NC * rows, :
                        ].rearrange("a b -> (a b)")
                    ],
                )

            # ---------- layer-1 projection prologue (chunked, AG pipelined)
            for ci, (t0, t1) in enumerate(CHUNKS):
                for tloc in range(t1 - t0):
                    tg = t0 + tloc
                    proj_tile(1, [x_fm_t[:, bass.ts(tg, 128)]], tloc, ci)
                ag_chunk(1, ci)

            # ---------- layers
            for l in (1, 2, 3):
                for ci, (t0, t1) in enumerate(CHUNKS):
                    for tloc in range(t1 - t0):
                        t = t0 + tloc
                        ea, eb, ec = eca[t], ecb[t], ECl[t]
                        KA_t, KB_t = ea * 128, eb * 128
                        ET_t = ec * 128
                        A = gpool.tile([128, ECMAX, TE], F16, tag="A")
                        nc.gpsimd.dma_gather(
                            out_ap=A[:, :ea, :],
                            in_ap=fs_full[l][:SPLIT, :],
                            idxs_ap=idxA_t[:, offA[t] : offA[t + 1]],
                            num_idxs=KA_t,
                            num_idxs_reg=KA_t,
                            elem_size=TE,
                            queue_num=(2 * t) % 4,
                        )
                        if eb:
                            nc.gpsimd.dma_gather(
                                out_ap=A[:, ea : ea + eb, :],
                                in_ap=fs_full[l][SPLIT:, :],
                                idxs_ap=idxB_t[:, offB[t] : offB[t + 1]],
                                num_idxs=KB_t,
                                num_idxs_reg=KB_t,
                                elem_size=TE,
                                queue_num=(2 * t + 1) % 4,
                            )
                        snm_t = opool.tile([128, ECMAX * 128], F8, tag="snm")
                        nc.sync.dma_start(
                            snm_t[:, :ET_t], snm_all[:, offO[t] : offO[t + 1]]
                        )
                        st_t = opool.tile([128, ECMAX * 128], F8, tag="st")
                        nc.sync.dma_start(
                            st_t[:, :ET_t], st_all[:, offO[t] : offO[t + 1]]
                        )
                        # z = fd[dst] + fs, chunk-pairs in PSUM; leaky -> C
                        C = epool.tile([128, ECMAX, HD], F16, tag="C")
                        for j0 in range(0, ec, 2):
                            jn = min(2, ec - j0)
                            zps = psZ.tile([128, 2, HD], F32, space="PSUM", tag="zps", name="zps")
                            for j in range(j0, j0 + jn):
                                nc.tensor.matmul(
                                    zps[:, j - j0, :],
                                    lhsT=snm_t[:, bass.ts(j, 128)],
                                    rhs=fd_of[l][:, t, :],
                                    start=True, stop=False,
                                )
                                nc.tensor.matmul(
                                    zps[:, j - j0, :],
                                    lhsT=ident8_t[:],
                                    rhs=A[:, j, :HD],
                                    start=False, stop=True,
                                )
                            nc.scalar.activation(
                                C[:, j0 : j0 + jn, :],
                                zps[:, :jn, :],
                                mybir.ActivationFunctionType.Prelu,
                                alpha=NEG_GAT,
                            )
                        # logits
                        AM = epool.tile([128, ECMAX, HD], F16, tag="AM")
                        nc.vector.tensor_tensor(
                            out=AM[:, :ec, :], in0=C[:, :ec, :],
                            in1=a_t[l][:, None, :].to_broadcast([128, ec, HD]),
                            op=mybir.AluOpType.mult,
                        )
                        logit = spool.tile([128, ECMAX, H], F32, tag="logit")
                        nc.vector.tensor_reduce(
                            out=logit[:, :ec, :],
                            in_=AM[:, :ec, :].rearrange("p a (h d) -> p a h d", h=H),
                            axis=mybir.AxisListType.X,
                            op=mybir.AluOpType.add,
                        )
                        EFX = epool.tile([128, ECMAX, HD + H], F16, tag="EFX")
                        ex = EFX[:, :ec, HD : HD + H]
                        nc.scalar.activation(
                            ex, logit[:, :ec, :], mybir.ActivationFunctionType.Exp
                        )
                        nc.vector.tensor_tensor(
                            out=EFX[:, :ec, :HD].rearrange("p a (h d) -> p a h d", h=H),
                            in0=A[:, :ec, :HD].rearrange("p a (h d) -> p a h d", h=H),
                            in1=ex[:, :, :, None].to_broadcast([128, ec, H, D]),
                            op=mybir.AluOpType.mult,
                        )
                        ps_ud = psB.tile([128, HD + H], F32, space="PSUM", tag="ps_ud", name="ps_ud")
                        for j in range(ec):
                            nc.tensor.matmul(
                                ps_ud[:], lhsT=st_t[:, bass.ts(j, 128)],
                                rhs=EFX[:, j, :],
                                start=(j == 0), stop=(j == ec - 1),
                            )
                        dmax = spool.tile([128, H], F32, tag="dmax")
                        nc.vector.tensor_scalar_max(dmax[:], ps_ud[:, HD : HD + H], 1e-9)
                        rden = spool.tile([128, H], F32, tag="rden")
                        nc.vector.reciprocal(rden[:], dmax[:])
                        hm = spool.tile([128, H, D], F16, tag="hm")
                        nc.vector.tensor_tensor(
                            out=hm[:],
                            in0=ps_ud[:, :HD].rearrange("p (h d) -> p h d", h=H),
                            in1=rden[:, :, None].to_broadcast([128, H, D]),
                            op=mybir.AluOpType.mult,
                        )
                        if l < 3:
                            h_sb = ppool.tile([128, HPAD], F16, tag="h_sb")
                            nc.vector.tensor_tensor(
                                out=h_sb[:, :HD],
                                in0=hm[:].rearrange("p h d -> p (h d)"),
                                in1=b_t[l][:],
                                op=mybir.AluOpType.add,
                            )
                            nc.vector.memset(h_sb[:, HD:], 0.0)
                            nc.sync.dma_start(
                                h_dram[(l, ci)][bass.ts(tloc, 128), :], h_sb[:]
                            )
                        else:
                            rhs65 = ppool.tile([128, 65], F16, tag="rhs65")
                            t01 = spool.tile([128, D], F16, tag="t01")
                            nc.vector.tensor_tensor(
                                out=t01[:], in0=hm[:, 0, :], in1=hm[:, 1, :],
                                op=mybir.AluOpType.add,
                            )
                            t012 = spool.tile([128, D], F16, tag="t012")
                            nc.vector.tensor_tensor(
                                out=t012[:], in0=t01[:], in1=hm[:, 2, :],
                                op=mybir.AluOpType.add,
                            )
                            nc.vector.scalar_tensor_tensor(
                                out=rhs65[:, :D], in0=t012[:], scalar=1.0 / H,
                                in1=b3m_t[:], op0=mybir.AluOpType.mult,
                                op1=mybir.AluOpType.add,
                            )
                            nc.vector.memset(rhs65[:, 64:65], 1.0)
                            nc.tensor.matmul(
                                gp_ps[:], lhsT=gsel_t[:, bass.ts(t, G)], rhs=rhs65[:],
                                start=(t == 0), stop=(t == NT - 1),
                            )
                    # ---- after finishing chunk ci of layer l: pipeline the
                    # next layer's projection + allgather for this chunk
                    if l < 3:
                        rows = chunk_rows[ci]
                        hT1 = hTpool.tile([128, rows], F16, tag="hT1", name="hT1")
                        nc.sync.dma_start(
                            hT1[:], h_dram[(l, ci)][:, 0:128], transpose=True
                        )
                        hT2 = hTpool.tile([128, rows], F16, tag="hT2", name="hT2")
                        nc.sync.dma_start(
                            hT2[:], h_dram[(l, ci)][:, 128:256], transpose=True
                        )
                        for tloc in range(t1 - t0):
                            proj_tile(
                                l + 1,
                                [
                                    hT1[:, bass.ts(tloc, 128)],
                                    hT2[:64, bass.ts(tloc, 128)],
                                ],
                                tloc, ci,
                            )
                        ag_chunk(l + 1, ci)

            # ================= epilogue
            part_sb = spool.tile([G, 65], F32, tag="part_sb")
            nc.vector.tensor_copy(part_sb[:], gp_ps[:])
            nc.sync.dma_start(partials[:], part_sb[:])
            nc.gpsimd.collective_compute(
                "AllReduce",
                mybir.AluOpType.add,
                replica_groups=groups,
                ins=[partials[:]],
                outs=[partials_red[:]],
            )
            red_sb = spool.tile([G, 65], F32, tag="red_sb")
            nc.sync.dma_start(red_sb[:], partials_red[:])

            xg = spool.tile([G, 128], F16, tag="xg")
            rc = spool.tile([G, 1], F32, tag="rc")
            cnt1 = spool.tile([G, 1], F32, tag="cnt1")
            nc.vector.tensor_scalar_max(cnt1[:], red_sb[:, 64:65], 1.0)
            nc.vector.reciprocal(rc[:], cnt1[:])
            nc.vector.tensor_tensor(
                out=xg[:, :64], in0=red_sb[:, :64],
                in1=rc[:].to_broadcast([G, 64]), op=mybir.AluOpType.mult,
            )

            px_ps = psA.tile([G, 96], F32, space="PSUM", tag="psP", name="px_ps")
            Wex_t = spool.tile([64, 32], F16, tag="Wex_t")
            nc.sync.dma_start(Wex_t[:], Wex[:])
            for i in range(3):
                pT = spool.tile([64, G], F16, tag=f"pT{i}", name=f"pT{i}")
                nc.sync.dma_start(pT[:], p123T[i][:])
                nc.tensor.matmul(
                    px_ps[:, 32 * i : 32 * i + 32], lhsT=pT[:], rhs=Wex_t[:],
                    start=True, stop=True,
                )
            bex_t = spool.tile([G, 96], F16, tag="bex_t")
            nc.sync.dma_start(bex_t[:], bex96_rep[:])
            pxc = spool.tile([G, 96], F16, tag="pxc")
            nc.vector.tensor_tensor(
                out=pxc[:], in0=px_ps[:], in1=bex_t[:], op=mybir.AluOpType.add
            )

            def small_mm(x_sb, pdim, w_t, b_t_, odim, leaky, out_ap, out_f32=False):
                tp = psT.tile([128, 128], F16, space="PSUM", tag="tp", name="ep_tp")
                nc.tensor.transpose(tp[:pdim, :G], x_sb[:, :pdim], ident_t[:G, :G])
                xT = spool.tile([128, G], F16, tag="ep_xT")
                nc.scalar.copy(xT[:pdim, :], tp[:pdim, :G])
                mm = psA.tile([G, 64], F32, space="PSUM", tag="psP", name="ep_mm")
                nc.tensor.matmul(
                    mm[:, :odim], lhsT=xT[:pdim, :], rhs=w_t[:], start=True, stop=True
                )
                tmp = spool.tile([G, 64], F32 if out_f32 else F16, tag="ep_tmp")
                nc.vector.tensor_tensor(
                    out=tmp[:, :odim], in0=mm[:, :odim], in1=b_t_[:],
                    op=mybir.AluOpType.add,
                )
                if leaky:
                    nc.vector.scalar_tensor_tensor(
                        out=out_ap, in0=tmp[:, :odim], scalar=NEG,
                        in1=tmp[:, :odim], op0=mybir.AluOpType.mult,
                        op1=mybir.AluOpType.max,
                    )
                else:
                    nc.vector.tensor_copy(out_ap, tmp[:, :odim])

            Wpat_t = spool.tile([96, 64], F16, tag="Wpat_t")
            nc.sync.dma_start(Wpat_t[:], Wpat[:])
            bpat_t = spool.tile([G, 64], F16, tag="bpat_t")
            nc.sync.dma_start(bpat_t[:], bpat_rep[:])
            small_mm(pxc, 96, Wpat_t, bpat_t, 64, True, xg[:, 64:128])

            Wc1_t = spool.tile([128, 64], F16, tag="Wc1_t")
            nc.sync.dma_start(Wc1_t[:], Wc1[:])
            bc1_t = spool.tile([G, 64], F16, tag="bc1_t")
            nc.sync.dma_start(bc1_t[:], bc1_rep[:])
            h1 = spool.tile([G, 64], F16, tag="ep_h1")
            small_mm(xg, 128, Wc1_t, bc1_t, 64, True, h1[:])

            Wc2_t = spool.tile([64, 32], F16, tag="Wc2_t")
            nc.sync.dma_start(Wc2_t[:], Wc2[:])
            bc2_t = spool.tile([G, 32], F16, tag="bc2_t")
            nc.sync.dma_start(bc2_t[:], bc2_rep[:])
            h2 = spool.tile([G, 32], F16, tag="ep_h2")
            small_mm(h1, 64, Wc2_t, bc2_t, 32, True, h2[:])

            Wc3_t = spool.tile([32, 2], F16, tag="Wc3_t")
            nc.sync.dma_start(Wc3_t[:], Wc3[:])
            bc3_t = spool.tile([G, 2], F16, tag="bc3_t")
            nc.sync.dma_start(bc3_t[:], bc3_rep[:])
            h3 = spool.tile([G, 2], F32, tag="ep_h3")
            small_mm(h2, 32, Wc3_t, bc3_t, 2, False, h3[:], out_f32=True)
            nc.sync.dma_start(out[:], h3[:])

    nc.finalize()
    return nc


# ---------------------------------------------------------------- entry point

def _run(inputs, trace=False, **trace_kwargs):
    cfg = _derive(_default_cfg())
    in_maps, meta = prep_host(inputs, cfg)
    nc = build_gat(cfg, meta)
    res = run_bass_kernel_spmd(
        nc, in_maps, core_ids=list(range(cfg["NC"])), trace=trace, **trace_kwargs
    )
    return np.asarray(res.results[0]["out"], np.float32), res


def kernel(**inputs):
    out, _ = _run(inputs, trace=False)
    return out
